# revision 1
# baseline (speedup 1.0000x reference)
"""BiLSTM-CRF loss kernel for 8 Trainium2 NeuronCores.

Math (per sequence):
  NLL = log Z - gold
  log Z:  forward algorithm over L=1024 steps, T=32 tags.
  gold:   score of the labelled path.

Device formulation (linear domain, periodically rescaled):
  a_{l+1} = diag(exp(f_l)) @ E^T @ a_l          E[j,i] = exp(trans[i,j])
  Z = sum_i a_L[i] * exp(trans[STOP, i])
  The gold score is the same recurrence with exp(f_l) masked to the
  labelled tag (one-hot), so it shares all device machinery.

Each core processes 128 sequences (pure batch data-parallel).  Four
independent chains ride the 128 SBUF partitions as 4 slices of 32 tags:
  slice 0: Z forward        slice 1: gold forward
  slice 2: Z backward       slice 3: gold backward
Forward chains cover steps 0..511, backward chains cover 1023..512 and
the halves are joined with one extra matmul.  One 128x128
block-diagonal bf16 matmul + one DVE tensor-tensor multiply advance all
four chains by one step.  To hide the PE->PSUM->DVE latency the 128
sequences are further split into two independent half-chains (64 seqs
each) that software-pipeline against each other; 512 supersteps total.

Host-side staging only reorders/masks the inputs: feats are laid out as
[(slice, tag), superstep, seq] bf16, with the gold slices replaced by
feats-where-tag-matches / -inf elsewhere.  exp() happens on device.
"""

import sys

sys.path.insert(0, "/opt/trn_rl_repo")

import numpy as np
import ml_dtypes

B, L, T = 1024, 1024, 32
START, STOP = 30, 31
NCORES = 8
BS = B // NCORES          # sequences per core
HB = BS // 2              # sequences per half-chain (legacy name)
GROUPS = [(0, 64), (64, 64)]             # (seq offset, size) per chain group
S = L // 2                # supersteps
CH = 64                   # supersteps per DMA/exp chunk
NCH = S // CH
RESCALE_EVERY = 128       # supersteps between rescales
MASK_NEG = -60000.0       # exp(MASK_NEG + bias) == 0 in fp32/bf16
MU_Z = 3.88               # mean per-step log-growth of the Z chains
MU_G = 0.0                # mean per-step log-growth of the gold chains

_compiled = None


def _build_nc():
    import concourse.bacc as bacc
    import concourse.tile as tile
    import concourse.mybir as mybir
    import concourse.masks as masks
    from concourse.bass import AP

    fp32 = mybir.dt.float32
    bf16 = mybir.dt.bfloat16

    nc = bacc.Bacc(
        "TRN2",
        target_bir_lowering=False,
        debug=False,
        enable_asserts=False,
        num_devices=NCORES,
    )
    staged_d = nc.dram_tensor("staged", [128, S * 128], bf16, kind="ExternalInput").ap()
    trans_d = nc.dram_tensor("trans", [T, T], fp32, kind="ExternalInput").ap()
    out_d = nc.dram_tensor("out", [BS, 1], fp32, kind="ExternalOutput").ap()

    from contextlib import ExitStack

    with tile.TileContext(nc) as tc, ExitStack() as ctx:
        singles = ctx.enter_context(tc.tile_pool(name="singles", bufs=1))
        st_pool = ctx.enter_context(tc.tile_pool(name="staged", bufs=2))
        fx_pool = ctx.enter_context(tc.tile_pool(name="fexp", bufs=2))
        rhs_pool = ctx.enter_context(tc.tile_pool(name="rhs", bufs=6))
        ps_pool = ctx.enter_context(tc.tile_pool(name="psum", bufs=2, space="PSUM"))
        psb_pool = ctx.enter_context(tc.tile_pool(name="psumb", bufs=2, space="PSUM"))
        sm_pool = ctx.enter_context(tc.tile_pool(name="small", bufs=2))

        # chunk-0 staged DMA first, so it isn't queued behind the constant
        # loads on the HWDGE FIFO
        st0 = st_pool.tile([128, 4 * 128], bf16, tag="st", name="st_0")
        nc.sync.dma_start(out=st0[:], in_=staged_d[:, 0 : 4 * 128])

        # ---- constants -------------------------------------------------
        trans_rep = singles.tile([128, T], fp32, tag="trans_rep")
        for k in range(4):
            # split across SWDGE and HWDGE queues so the four replication
            # DMAs run in parallel instead of serializing on one FIFO
            eng = nc.gpsimd if k % 2 == 0 else nc.sync
            eng.dma_start(out=trans_rep[32 * k : 32 * (k + 1), :], in_=trans_d)
        # E_rep[32k+i, j] = exp(trans[i, j])   (lhsT for the backward blocks)
        e_rep = singles.tile([128, T], bf16, tag="e_rep")
        nc.scalar.activation(e_rep[:], trans_rep[:], mybir.ActivationFunctionType.Exp)
        # E_repT[32k+j, i] = exp(trans[i, j])  (lhsT for the forward blocks)
        e_rept = singles.tile([128, T], bf16, tag="e_rept")
        nc.vector.transpose(e_rept[:], e_rep[:])

        # W1: block-diag stationary [(zf, gf) -> E^T-form, (zb, gb) -> E-form]
        w1 = singles.tile([128, 128], bf16, tag="w1")
        nc.vector.memset(w1[:], 0.0)
        nc.vector.tensor_copy(w1[0:32, 0:32], e_rept[0:32, :])
        nc.vector.tensor_copy(w1[32:64, 32:64], e_rept[32:64, :])
        nc.vector.tensor_copy(w1[64:96, 64:96], e_rep[64:96, :])
        nc.vector.tensor_copy(w1[96:128, 96:128], e_rep[96:128, :])

        # W2: final join; fwd state rows -> bwd-aligned output partitions
        w2 = singles.tile([128, 128], bf16, tag="w2")
        nc.vector.memset(w2[:], 0.0)
        nc.vector.tensor_copy(w2[0:32, 64:96], e_rept[0:32, :])
        nc.vector.tensor_copy(w2[32:64, 96:128], e_rept[32:64, :])

        ident = singles.tile([128, 128], bf16, tag="ident")
        masks.make_identity(nc, ident[:])

        # per-partition bias for the bulk exp: exp(feat - mu)
        bias = singles.tile([128, 1], fp32, tag="bias")
        nc.vector.memset(bias[0:32, :], -MU_Z)
        nc.vector.memset(bias[32:64, :], -MU_G)
        nc.vector.memset(bias[64:96, :], -MU_Z)
        nc.vector.memset(bias[96:128, :], -MU_G)

        # rescale log accumulators [seq-in-group, slice], one per chain group
        accs = []
        for h, (off, gsz) in enumerate(GROUPS):
            a = singles.tile([gsz, 4], fp32, tag=f"acc{h}")
            nc.vector.memset(a[:], 0.0)
            accs.append(a)

        # ---- chunk loading --------------------------------------------
        # small leading chunks so the chains start early; steady-state CH
        chunk_sched = [(0, 4), (4, 12), (16, 48)]
        while chunk_sched[-1][0] + chunk_sched[-1][1] < S:
            c0 = chunk_sched[-1][0] + chunk_sched[-1][1]
            chunk_sched.append((c0, min(CH, S - c0)))

        def load_chunk(c0, clen, st=None):
            if st is None:
                st = st_pool.tile([128, clen * 128], bf16, tag="st", name=f"st_{c0}")
                nc.sync.dma_start(
                    out=st[:], in_=staged_d[:, c0 * 128 : (c0 + clen) * 128]
                )
            fx = fx_pool.tile([128, clen * 128], bf16, tag="fx", name=f"fx_{c0}")
            nc.scalar.activation(
                fx[:], st[:], mybir.ActivationFunctionType.Exp, bias=bias[:]
            )
            return fx

        fx = load_chunk(*chunk_sched[0], st=st0)

        # ---- chain init ------------------------------------------------
        expstop = singles.tile([128, 1], fp32, tag="expstop")
        nc.vector.tensor_copy(expstop[:], e_rept[:, STOP : STOP + 1])

        rhs = []
        for h, (off, gsz) in enumerate(GROUPS):
            r = rhs_pool.tile([128, gsz], bf16, tag=f"rhs{h}", name=f"rhs{h}_i")
            nc.vector.memset(r[:], 0.0)
            for sl in (0, 32):
                nc.gpsimd.affine_select(
                    out=r[sl : sl + 32, :], in_=r[sl : sl + 32, :],
                    pattern=[[0, gsz]],
                    compare_op=mybir.AluOpType.not_equal, fill=1.0,
                    base=-START, channel_multiplier=1,
                )
            # backward init: c_1023 = fexp_1023 * expstop  (slot 0, this group)
            nc.scalar.mul(
                r[64:128, :], fx[64:128, off : off + gsz], expstop[64:128, :]
            )
            rhs.append(r)

        # ---- rescale ---------------------------------------------------
        def rescale(h, state, s):
            gsz = GROUPS[h][1]
            pst = psb_pool.tile([gsz, 128], bf16, tag="psx", name=f"pst{h}_{s}")
            nc.tensor.matmul(pst[:], state[:], ident[:, 0:128], is_transpose=True)
            pst3 = pst[:].rearrange("p (s t) -> p s t", t=32)
            mx = sm_pool.tile([gsz, 4], fp32, tag="mx")
            nc.vector.tensor_reduce(
                mx[:], pst3, axis=mybir.AxisListType.X, op=mybir.AluOpType.max
            )
            lg = sm_pool.tile([gsz, 4], fp32, tag="lg")
            nc.scalar.activation(lg[:], mx[:], mybir.ActivationFunctionType.Ln)
            nc.vector.tensor_add(accs[h][:], accs[h][:], lg[:])
            rcp = sm_pool.tile([gsz, 4], fp32, tag="rcp")
            nc.vector.reciprocal(rcp[:], mx[:])
            rcp_b = AP(
                tensor=rcp[:].tensor,
                offset=rcp[:].offset,
                ap=[rcp[:].ap[0], rcp[:].ap[1], [0, 32]],
            )
            st2 = sm_pool.tile([gsz, 128], bf16, tag="st2")
            nc.vector.tensor_mul(
                st2[:].rearrange("p (s t) -> p s t", t=32), pst3, rcp_b
            )
            psb = psb_pool.tile([128, gsz], bf16, tag="psx", name=f"psb{h}_{s}")
            nc.tensor.matmul(psb[:], st2[:], ident[0:gsz, 0:gsz], is_transpose=True)
            out = rhs_pool.tile([128, gsz], bf16, tag=f"rhs{h}", name=f"rhsr{h}_{s}")
            nc.vector.tensor_copy(out[:], psb[:])
            return out

        # ---- main loop -------------------------------------------------
        chunk_idx = 0
        for s in range(S):
            if s >= chunk_sched[chunk_idx][0] + chunk_sched[chunk_idx][1]:
                chunk_idx += 1
                fx = load_chunk(*chunk_sched[chunk_idx])
            sl = s - chunk_sched[chunk_idx][0]
            for h, (off, gsz) in enumerate(GROUPS):
                psh = ps_pool.tile([128, gsz], fp32, tag=f"ps{h}", name=f"ps{h}_{s}")
                nc.tensor.matmul(psh[:], w1[:], rhs[h][:], start=True, stop=True)
                nrhs = rhs_pool.tile([128, gsz], bf16, tag=f"rhs{h}", name=f"rhs{h}_{s}")
                fsl = fx[:, sl * 128 + off : sl * 128 + off + gsz]
                if s == 0:
                    nc.vector.tensor_mul(nrhs[0:64, :], psh[0:64, :], fsl[0:64, :])
                    nc.vector.tensor_copy(nrhs[64:128, :], rhs[h][64:128, :])
                else:
                    nc.vector.tensor_mul(nrhs[:], psh[:], fsl)
                rhs[h] = nrhs
            if s % RESCALE_EVERY == RESCALE_EVERY - 1:
                for h in range(len(GROUPS)):
                    rhs[h] = rescale(h, rhs[h], s)

        # ---- final join ------------------------------------------------
        for h, (off, gsz) in enumerate(GROUPS):
            psf = ps_pool.tile([128, gsz], fp32, tag=f"ps{h}", name=f"psf{h}")
            nc.tensor.matmul(psf[:], w2[:], rhs[h][:], start=True, stop=True)
            # TT operands must share partitions; psf/rhs slices are on 64:128,
            # so allocate a [128, gsz] tile and use its upper half.
            prod128 = sm_pool.tile([128, gsz], bf16, tag="prod128", name=f"prod{h}")
            nc.vector.tensor_mul(
                prod128[64:128, :], psf[64:128, :], rhs[h][64:128, :]
            )
            pst = psb_pool.tile([gsz, 64], bf16, tag="psx", name=f"pstf{h}")
            nc.tensor.matmul(
                pst[:], prod128[64:128, :], ident[64:128, 64:128],
                is_transpose=True, tile_position=(64, 0),
            )
            zg = sm_pool.tile([gsz, 2], fp32, tag="zg")
            nc.vector.tensor_reduce(
                zg[:],
                pst[:].rearrange("p (s t) -> p s t", t=32),
                axis=mybir.AxisListType.X,
                op=mybir.AluOpType.add,
            )
            lzg = sm_pool.tile([gsz, 2], fp32, tag="lzg")
            nc.scalar.activation(lzg[:], zg[:], mybir.ActivationFunctionType.Ln)
            # nll = (lz - lg) + (acc0 + acc2 - acc1 - acc3) + L * (MU_Z - MU_G)
            t0 = sm_pool.tile([gsz, 1], fp32, tag="t0")
            nc.vector.tensor_sub(t0[:], lzg[:, 0:1], lzg[:, 1:2])
            t1 = sm_pool.tile([gsz, 1], fp32, tag="t1")
            nc.vector.tensor_add(t1[:], accs[h][:, 0:1], accs[h][:, 2:3])
            t2 = sm_pool.tile([gsz, 1], fp32, tag="t2")
            nc.vector.tensor_add(t2[:], accs[h][:, 1:2], accs[h][:, 3:4])
            t3 = sm_pool.tile([gsz, 1], fp32, tag="t3")
            nc.vector.tensor_sub(t3[:], t1[:], t2[:])
            t4 = sm_pool.tile([gsz, 1], fp32, tag="t4")
            nc.vector.tensor_add(t4[:], t0[:], t3[:])
            res_h = sm_pool.tile([gsz, 1], fp32, tag=f"res{h}")
            nc.vector.tensor_scalar_add(res_h[:], t4[:], float(L) * (MU_Z - MU_G))
            nc.sync.dma_start(out=out_d[off : off + gsz, :], in_=res_h[:])

    nc.compile()
    return nc


def _stage_core(feats_c, tags_c):
    """feats_c [128, 1024, 32] f32, tags_c [128, 1024] int -> [128, S*128] bf16."""
    ft = np.ascontiguousarray(feats_c.transpose(2, 1, 0))        # [t, l, b]
    mask = tags_c[None, :, :] == np.arange(T, dtype=tags_c.dtype)[:, None, None]
    # mask[t, b, l] -> want [t, l, b]
    mask = mask.transpose(0, 2, 1)
    gt = np.where(mask, ft, np.float32(MASK_NEG))
    staged = np.empty((4, T, S, BS), np.float32)
    staged[0] = ft[:, :S, :]
    staged[1] = gt[:, :S, :]
    staged[2] = ft[:, ::-1, :][:, :S, :]
    staged[3] = gt[:, ::-1, :][:, :S, :]
    return staged.reshape(128, S * BS).astype(ml_dtypes.bfloat16)


LAST_RESULTS = None


def kernel(feats, transitions, tags, _trace=False):
    global _compiled, LAST_RESULTS
    from concourse.bass_utils import run_bass_kernel_spmd

    feats = np.asarray(feats, dtype=np.float32)
    transitions = np.asarray(transitions, dtype=np.float32)
    tags = np.asarray(tags)

    if _compiled is None:
        _compiled = _build_nc()
    nc = _compiled

    in_maps = []
    for c in range(NCORES):
        sl = slice(c * BS, (c + 1) * BS)
        in_maps.append(
            {
                "staged": _stage_core(feats[sl], tags[sl]),
                "trans": transitions,
            }
        )
    res = run_bass_kernel_spmd(
        nc, in_maps, core_ids=list(range(NCORES)), trace=_trace
    )
    LAST_RESULTS = res
    out = np.concatenate([r["out"].reshape(BS) for r in res.results])
    return out.astype(np.float32)



# revision 4
# speedup vs baseline: 2.8588x; 2.8588x over previous
"""BiLSTM-CRF loss kernel for 8 Trainium2 NeuronCores.

Math (per sequence):
  NLL = log Z - gold
  log Z:  forward algorithm over L=1024 steps, T=32 tags.
  gold:   score of the labelled path.

Segmented rank-1 skeleton algorithm (device):
  Z = stop^T M_{L-1}...M_0 e_START with M_l = D_l X, X = exp(trans),
  D_l = diag(exp(f_l - mu)).  L is split into S=32 segments of K=32
  steps.  Products of >=16 consecutive M_l are numerically rank-1
  (Birkhoff contraction; the diagonals cancel in cross-ratios), so
  P_s ~= (P_s p)(q^T P_s)/(q^T P_s p).  Each segment's forward probe
  a_s = P_s p and backward probe w_s (with b_s = X^T w_s folded into
  the junctions) are independent vector recurrences of K steps, so the
  serial depth is K=32 supersteps instead of 512:

    log Z = L*mu + sum_{s=1}^{S-1} log(w_s . (X a_{s-1}))
                 - sum_{s=1}^{S-2} log(sum a_s)

  with p = e_START for s=0 and q = X[STOP,:] for s=S-1 (both exact).

  gold is not run through the recurrence at all:
    emit  = sum_l feats[b,l,tags[b,l]]      (host-gathered values,
                                             device reduction)
    trans = sum_{pairs} count[b,pair] * trans[pair]
                                            (host-counted pairs,
                                             device matmuls)

Layout per core (128 sequences): 64 chains (fwd+bwd per segment) ride
the 128 partitions as 8 "quads" of 4 segments x 32 tags.  Per superstep:
16 matmuls (one per quad per direction, 128 seq columns each) and 4
drain instructions (PSUM*fx -> SBUF multiply, merged across quads,
split DVE/Pool by sequence range).  exp(f - mu) runs on the Activation
engine in chunk pairs (low-k for fwd, high-k for bwd) so DMA, exp and
the main loop pipeline.
"""

import sys

sys.path.insert(0, "/opt/trn_rl_repo")

import numpy as np
import ml_dtypes

B, L, T = 1024, 1024, 32
START, STOP = 30, 31
NCORES = 8
BS = B // NCORES          # sequences per core
K = 32                    # steps per segment == supersteps
S = L // K                # segments
NQ = 8                    # quads (4 segments each)
MU = 3.88                 # per-step log-growth bias
DK = 2                    # k-chunk size for DMA/exp pipelining
NPAIR = K // DK           # chunk pairs
SD = 72                   # drain seq split: DVE gets cols [0,SD), Pool the rest

_compiled = None


def _build_nc():
    import concourse.bacc as bacc
    import concourse.tile as tile
    import concourse.mybir as mybir
    import concourse.masks as masks

    fp32 = mybir.dt.float32
    bf16 = mybir.dt.bfloat16
    Exp = mybir.ActivationFunctionType.Exp
    Ln = mybir.ActivationFunctionType.Ln
    mult = mybir.AluOpType.mult

    nc = bacc.Bacc(
        "TRN2",
        target_bir_lowering=False,
        debug=False,
        enable_asserts=False,
        num_devices=NCORES,
    )
    staged_d = nc.dram_tensor("staged", [128, K * NQ * 128], bf16, kind="ExternalInput").ap()
    gvals_d = nc.dram_tensor("gvals", [128, L], bf16, kind="ExternalInput").ap()
    counts_d = nc.dram_tensor("counts", [128, 8 * 128], fp32, kind="ExternalInput").ap()
    trans_d = nc.dram_tensor("trans", [T, T], fp32, kind="ExternalInput").ap()
    out_d = nc.dram_tensor("out", [1, BS], fp32, kind="ExternalOutput").ap()

    from contextlib import ExitStack

    with tile.TileContext(nc) as tc, ExitStack() as ctx:
        singles = ctx.enter_context(tc.tile_pool(name="singles", bufs=1))
        stg_pool = ctx.enter_context(tc.tile_pool(name="stg", bufs=3))
        fx_pool = ctx.enter_context(tc.tile_pool(name="fx", bufs=3))
        st_pool = ctx.enter_context(tc.tile_pool(name="st", bufs=2))
        ps_pool = ctx.enter_context(tc.tile_pool(name="ps", bufs=1, space="PSUM"))
        sm_pool = ctx.enter_context(tc.tile_pool(name="sm", bufs=2))

        # ---- input DMAs ------------------------------------------------
        # staged chunk pairs on the HWDGE (sync) queue, small tensors on
        # the SWDGE (gpsimd) queue so they land in parallel.
        stg_f = []
        stg_b = []
        for j in range(NPAIR):
            f = stg_pool.tile([128, DK * 1024], bf16, tag="stgf", name=f"stgf_{j}")
            nc.sync.dma_start(out=f[:], in_=staged_d[:, 2 * j * 1024 : (2 * j + 2) * 1024])
            b = stg_pool.tile([128, DK * 1024], bf16, tag="stgb", name=f"stgb_{j}")
            nc.sync.dma_start(
                out=b[:], in_=staged_d[:, (K - 2 - 2 * j) * 1024 : (K - 2 * j) * 1024]
            )
            stg_f.append(f)
            stg_b.append(b)

        trans_rep = singles.tile([128, T], fp32, tag="trans_rep")
        for kk in range(4):
            nc.gpsimd.dma_start(out=trans_rep[32 * kk : 32 * (kk + 1), :], in_=trans_d)
        # rhs for the count matmuls: rhs_tc[p, c] = trans.flat[c*128 + p]
        rhs_tc = singles.tile([128, 8], fp32, tag="rhs_tc")
        tflat = trans_d.rearrange("a (b o) -> (a b) o", o=1)
        for c in range(8):
            nc.gpsimd.dma_start(out=rhs_tc[:, c : c + 1], in_=tflat[c * 128 : (c + 1) * 128, :])
        counts_sb = singles.tile([128, 8 * 128], fp32, tag="counts_sb")
        nc.gpsimd.dma_start(out=counts_sb[:], in_=counts_d)
        g_sb = singles.tile([128, L], bf16, tag="g_sb")
        nc.gpsimd.dma_start(out=g_sb[:], in_=gvals_d)

        # ---- constants -------------------------------------------------
        # X_rep[32k+i, j] = X[i, j] = exp(trans[i, j])
        x_rep = singles.tile([128, T], bf16, tag="x_rep")
        nc.scalar.activation(x_rep[:], trans_rep[:], Exp)
        # Xt_rep[32k+j, i] = X[i, j]
        xt_rep = singles.tile([128, T], bf16, tag="xt_rep")
        nc.vector.transpose(xt_rep[:], x_rep[:])

        # W_f: blockdiag lhsT for out = X @ in  (lhsT[j,i] = X[i,j])
        w_f = singles.tile([128, 128], bf16, tag="w_f")
        nc.vector.memset(w_f[:], 0.0)
        for a in range(4):
            nc.vector.tensor_copy(w_f[32 * a : 32 * a + 32, 32 * a : 32 * a + 32],
                                  xt_rep[32 * a : 32 * a + 32, :])
        # W_b: blockdiag lhsT for out = X^T @ in  (lhsT[j,i] = X[j,i])
        w_b = singles.tile([128, 128], bf16, tag="w_b")
        nc.vector.memset(w_b[:], 0.0)
        for a in range(4):
            nc.vector.tensor_copy(w_b[32 * a : 32 * a + 32, 32 * a : 32 * a + 32],
                                  x_rep[32 * a : 32 * a + 32, :])

        # ones4[32a+t, a] = 1 (partition-sum per slice)
        ones4 = singles.tile([128, 4], bf16, tag="ones4")
        nc.gpsimd.memset(ones4[:], 0.0)
        for a in range(4):
            nc.gpsimd.memset(ones4[32 * a : 32 * a + 32, a : a + 1], 1.0)
        ones41 = singles.tile([4, 1], bf16, tag="ones41")
        nc.gpsimd.memset(ones41[:], 1.0)

        identf = singles.tile([128, 128], fp32, tag="identf")
        masks.make_identity(nc, identf[:])

        bias = singles.tile([128, 1], fp32, tag="bias")
        nc.vector.memset(bias[:], -MU)

        # stopcol[32k+j] = X[STOP, j] (fp32 for tensor_scalar)
        stopcol = singles.tile([128, 1], fp32, tag="stopcol")
        nc.vector.tensor_copy(stopcol[:], xt_rep[:, STOP : STOP + 1])

        # ---- gold: count matmuls + emit reduction ----------------------
        psg = ps_pool.tile([128, 512], fp32, tag="psg")
        for c in range(8):
            nc.tensor.matmul(
                psg[0:1, 0:128],
                rhs_tc[:, c : c + 1],
                counts_sb[:, c * 128 : (c + 1) * 128],
                start=(c == 0),
                stop=(c == 7),
            )
        emit = sm_pool.tile([128, 1], fp32, tag="emit")
        nc.vector.tensor_reduce(
            emit[:], g_sb[:].rearrange("p (o l) -> p o l", o=1),
            axis=mybir.AxisListType.X, op=mybir.AluOpType.add,
        )
        # emit^T -> psg[0:1, 128:256]
        nc.tensor.matmul(psg[0:1, 128:256], emit[:], identf[:], is_transpose=True)

        # ---- exp chunks ------------------------------------------------
        fx_f = [None] * NPAIR
        fx_b = [None] * NPAIR

        def emit_exp_pair(j):
            ff = fx_pool.tile([128, DK * 1024], bf16, tag="fxf", name=f"fxf_{j}")
            nc.scalar.activation(ff[:], stg_f[j][:], Exp, bias=bias[:])
            bb = fx_pool.tile([128, DK * 1024], bf16, tag="fxb", name=f"fxb_{j}")
            nc.scalar.activation(bb[:], stg_b[j][:], Exp, bias=bias[:])
            fx_f[j] = ff
            fx_b[j] = bb

        emit_exp_pair(0)

        # ---- state init ------------------------------------------------
        st_f = st_pool.tile([128, 1024], bf16, tag="stf", name="stf_init")
        nc.vector.memset(st_f[:], 1.0)
        nc.vector.memset(st_f[0:32, 0:128], 0.0)
        nc.vector.memset(st_f[START : START + 1, 0:128], 1.0)

        # bwd init: w = q * fx[K-1]; q = ones except slice (7,3) = stopvec
        st_b = st_pool.tile([128, 1024], bf16, tag="stb", name="stb_init")
        nc.vector.tensor_copy(st_b[:], fx_b[0][:, 1024:2048])
        nc.vector.tensor_scalar(
            st_b[96:128, 896:1024], fx_b[0][96:128, 1024 + 896 : 1024 + 1024],
            stopcol[96:128, :], None, mult,
        )

        # ---- main loop -------------------------------------------------
        def drains(dst, src_ps, fx, off, name):
            a3 = lambda ap: ap.rearrange("p (q s) -> p q s", s=128)
            d3, s3, f3 = a3(dst[:]), a3(src_ps[:]), a3(fx[:, off : off + 1024])
            nc.vector.tensor_mul(
                d3[:, :, 0:SD], s3[:, :, 0:SD], f3[:, :, 0:SD])
            nc.gpsimd.scalar_tensor_tensor(
                d3[:, :, SD:128], s3[:, :, SD:128], 1.0, f3[:, :, SD:128],
                op0=mult, op1=mult,
            )

        mega_f = ps_pool.tile([128, 1024], fp32, tag="mega_f")
        mega_b = ps_pool.tile([128, 1024], fp32, tag="mega_b")

        for k in range(K):
            j, of = k // DK, (k % DK) * 1024
            ob = (1 - (k % DK)) * 1024
            if k % DK == 0 and j + 1 < NPAIR:
                emit_exp_pair(j + 1)
            # fwd
            for q in range(NQ):
                nc.tensor.matmul(
                    mega_f[:, q * 128 : (q + 1) * 128], w_f,
                    st_f[:, q * 128 : (q + 1) * 128], start=True, stop=True,
                )
            nst_f = st_pool.tile([128, 1024], bf16, tag="stf", name=f"stf_{k}")
            drains(nst_f, mega_f, fx_f[j], of, f"f{k}")
            st_f = nst_f
            # bwd (k=0 is the init multiply above)
            if k > 0:
                for q in range(NQ):
                    nc.tensor.matmul(
                        mega_b[:, q * 128 : (q + 1) * 128], w_b,
                        st_b[:, q * 128 : (q + 1) * 128], start=True, stop=True,
                    )
                nst_b = st_pool.tile([128, 1024], bf16, tag="stb", name=f"stb_{k}")
                drains(nst_b, mega_b, fx_b[j], ob, f"b{k}")
                st_b = nst_b

        # ---- join ------------------------------------------------------
        # y = X a_{s-1} into the slice of segment s (shifted one slice up)
        for a in range(1, 4):
            nc.tensor.matmul(
                mega_b[32 * a : 32 * a + 32, 0:1024],
                xt_rep[32 * (a - 1) : 32 * a, :],
                st_f[32 * (a - 1) : 32 * a, 0:1024],
                start=True, stop=True,
                tile_position=(32 * (a - 1), 32 * a),
            )
        # slice 0 of quad q <- slice 3 of quad q-1 (q=0 wraps; excluded later)
        nc.tensor.matmul(
            mega_b[0:32, 128:1024], xt_rep[96:128, :], st_f[96:128, 0:896],
            start=True, stop=True, tile_position=(96, 0),
        )
        nc.tensor.matmul(
            mega_b[0:32, 0:128], xt_rep[96:128, :], st_f[96:128, 896:1024],
            start=True, stop=True, tile_position=(96, 0),
        )
        # z = w_s * y_{s-1}
        z_sb = sm_pool.tile([128, 1024], bf16, tag="z_sb")
        nc.vector.tensor_mul(z_sb[:], mega_b[:], st_b[:])
        # junction and normalizer partition-sums
        nc.tensor.matmul(mega_f[0:4, 0:1024], ones4, z_sb[:], start=True, stop=True)
        nc.tensor.matmul(
            mega_f[32:36, 0:1024], ones4, st_f[:], start=True, stop=True,
            tile_position=(0, 32),
        )
        lnj = sm_pool.tile([4, 1024], fp32, tag="lnj")
        nc.scalar.activation(lnj[:], mega_f[0:4, 0:1024], Ln)
        lnc = sm_pool.tile([4, 1024], fp32, tag="lnc")
        nc.scalar.activation(lnc[:], mega_f[32:36, 0:1024], Ln)
        # exclusions: s=0 has no junction; c only for s=1..S-2
        nc.vector.memset(lnc[3:4, 896:1024], 0.0)
        diff = sm_pool.tile([4, 1024], bf16, tag="diff")
        nc.vector.tensor_sub(diff[:], lnj[:], lnc[:])
        nc.vector.memset(diff[0:1, 0:128], 0.0)
        # logZ~ (per seq) = sum over (a, q): 8 accumulating matmuls
        for q in range(NQ):
            nc.tensor.matmul(
                psg[0:1, 256:384], ones41, diff[:, q * 128 : (q + 1) * 128],
                start=(q == 0), stop=(q == 7),
            )
        # nll = logZ~ + L*mu - emit - transpart
        t1 = sm_pool.tile([1, 128], fp32, tag="t1")
        nc.vector.tensor_sub(t1[:], psg[0:1, 256:384], psg[0:1, 128:256])
        t2 = sm_pool.tile([1, 128], fp32, tag="t2")
        nc.vector.tensor_sub(t2[:], t1[:], psg[0:1, 0:128])
        res = sm_pool.tile([1, 128], fp32, tag="res")
        nc.vector.tensor_scalar_add(res[:], t2[:], float(L) * MU)
        nc.sync.dma_start(out=out_d, in_=res[:])

    nc.compile()
    return nc


def _stage_core(feats_c, tags_c):
    """feats_c [128, 1024, 32] f32, tags_c [128, 1024] int -> dict of arrays."""
    bf16 = ml_dtypes.bfloat16
    # staged[32a+t, k, q, b] = feats_c[b, q*128 + a*32 + k, t]
    f = np.ascontiguousarray(feats_c.transpose(1, 2, 0))  # [l, t, b]
    f = f.reshape(NQ, 4, K, T, BS)                        # [q, a, k, t, b]
    staged = np.ascontiguousarray(f.transpose(1, 3, 2, 0, 4)).reshape(128, K * NQ * BS)
    # gathered emission values
    g = np.take_along_axis(feats_c, tags_c[:, :, None].astype(np.int64), axis=2)[:, :, 0]
    # transition pair counts: pair = to*32 + from over (START+tags, tags+STOP)
    pad_start = np.concatenate(
        [np.full((BS, 1), START, tags_c.dtype), tags_c], axis=1)
    pad_stop = np.concatenate(
        [tags_c, np.full((BS, 1), STOP, tags_c.dtype)], axis=1)
    pair = (pad_stop.astype(np.int64) * T + pad_start.astype(np.int64))  # [BS, L+1]
    cnt = np.zeros((BS, T * T), np.float32)
    np.add.at(cnt, (np.arange(BS)[:, None], pair), 1.0)
    counts = np.ascontiguousarray(
        cnt.T.reshape(8, 128, BS).transpose(1, 0, 2)).reshape(128, 8 * BS)
    return {
        "staged": staged.astype(bf16),
        "gvals": g.astype(bf16),
        "counts": counts,
    }


LAST_RESULTS = None


def kernel(feats, transitions, tags, _trace=False):
    global _compiled, LAST_RESULTS
    from concourse.bass_utils import run_bass_kernel_spmd

    feats = np.asarray(feats, dtype=np.float32)
    transitions = np.asarray(transitions, dtype=np.float32)
    tags = np.asarray(tags)

    if _compiled is None:
        _compiled = _build_nc()
    nc = _compiled

    in_maps = []
    for c in range(NCORES):
        sl = slice(c * BS, (c + 1) * BS)
        m = _stage_core(feats[sl], tags[sl])
        m["trans"] = transitions
        in_maps.append(m)
    res = run_bass_kernel_spmd(
        nc, in_maps, core_ids=list(range(NCORES)), trace=_trace
    )
    LAST_RESULTS = res
    out = np.concatenate([r["out"].reshape(BS) for r in res.results])
    return out.astype(np.float32)


# revision 10
# speedup vs baseline: 2.9390x; 1.0281x over previous
"""BiLSTM-CRF loss kernel for 8 Trainium2 NeuronCores.

Math (per sequence):
  NLL = log Z - gold
  log Z:  forward algorithm over L=1024 steps, T=32 tags.
  gold:   score of the labelled path.

Segmented rank-1 skeleton algorithm (device):
  Z = stop^T M_{L-1}...M_0 e_START with M_l = D_l X, X = exp(trans),
  D_l = diag(exp(f_l - mu)).  L is split into S=32 segments of K=32
  steps.  Products of >=16 consecutive M_l are numerically rank-1
  (Birkhoff contraction; the diagonals cancel in cross-ratios), so
  P_s ~= (P_s p)(q^T P_s)/(q^T P_s p).  Each segment's forward probe
  a_s = P_s p and backward probe w_s (with b_s = X^T w_s folded into
  the junctions) are independent vector recurrences of K steps, so the
  serial depth is K=32 supersteps instead of 512:

    log Z = L*mu + sum_{s=1}^{S-1} log(w_s . (X a_{s-1}))
                 - sum_{s=1}^{S-2} log(sum a_s)

  with p = e_START for s=0 and q = X[STOP,:] for s=S-1 (both exact).

  gold is not run through the recurrence at all:
    emit  = sum_l feats[b,l,tags[b,l]]      (host-gathered values,
                                             device reduction)
    trans = sum_{pairs} count[b,pair] * trans[pair]
                                            (host-counted pairs,
                                             device matmuls)

Layout per core (128 sequences): 64 chains (fwd+bwd per segment) ride
the 128 partitions as 8 "quads" of 4 segments x 32 tags.  Per superstep:
16 matmuls (one per quad per direction, 128 seq columns each) and 4
drain instructions (PSUM*fx -> SBUF multiply, merged across quads,
split DVE/Pool by sequence range).  exp(f - mu) runs on the Activation
engine in chunk pairs (low-k for fwd, high-k for bwd) so DMA, exp and
the main loop pipeline.
"""

import sys

sys.path.insert(0, "/opt/trn_rl_repo")

import numpy as np
import ml_dtypes

B, L, T = 1024, 1024, 32
START, STOP = 30, 31
NCORES = 8
BS = B // NCORES          # sequences per core
K = 32                    # steps per segment == supersteps
S = L // K                # segments
NQ = 8                    # quads (4 segments each)
MU = 3.88                 # per-step log-growth bias
DK = 2                    # k-chunk size for DMA/exp pipelining
NPAIR = K // DK           # chunk pairs
SD = 72                   # drain seq split: DVE gets cols [0,SD), Pool the rest

_compiled = None


def _build_nc():
    import concourse.bacc as bacc
    import concourse.tile as tile
    import concourse.mybir as mybir
    import concourse.masks as masks

    fp32 = mybir.dt.float32
    bf16 = mybir.dt.bfloat16
    Exp = mybir.ActivationFunctionType.Exp
    Ln = mybir.ActivationFunctionType.Ln
    mult = mybir.AluOpType.mult

    nc = bacc.Bacc(
        "TRN2",
        target_bir_lowering=False,
        debug=False,
        enable_asserts=False,
        num_devices=NCORES,
    )
    staged_d = nc.dram_tensor("staged", [128, K * NQ * 128], bf16, kind="ExternalInput").ap()
    gvals_d = nc.dram_tensor("gvals", [128, L], bf16, kind="ExternalInput").ap()
    counts_d = nc.dram_tensor("counts", [128, 8 * 128], fp32, kind="ExternalInput").ap()
    trans_d = nc.dram_tensor("trans", [T, T], fp32, kind="ExternalInput").ap()
    out_d = nc.dram_tensor("out", [1, BS], fp32, kind="ExternalOutput").ap()

    from contextlib import ExitStack

    with tile.TileContext(nc) as tc, ExitStack() as ctx:
        singles = ctx.enter_context(tc.tile_pool(name="singles", bufs=1))
        stg_pool = ctx.enter_context(tc.tile_pool(name="stg", bufs=3))
        fx_pool = ctx.enter_context(tc.tile_pool(name="fx", bufs=3))
        st_pool = ctx.enter_context(tc.tile_pool(name="st", bufs=2))
        ps_pool = ctx.enter_context(tc.tile_pool(name="ps", bufs=1, space="PSUM"))
        sm_pool = ctx.enter_context(tc.tile_pool(name="sm", bufs=2))

        # ---- input DMAs ------------------------------------------------
        # staged chunk pairs on the HWDGE (sync) queue, small tensors on
        # the SWDGE (gpsimd) queue so they land in parallel.
        stg_f = []
        stg_b = []
        for j in range(NPAIR):
            f = stg_pool.tile([128, DK * 1024], bf16, tag="stgf", name=f"stgf_{j}")
            nc.sync.dma_start(out=f[:], in_=staged_d[:, 2 * j * 1024 : (2 * j + 2) * 1024])
            b = stg_pool.tile([128, DK * 1024], bf16, tag="stgb", name=f"stgb_{j}")
            nc.sync.dma_start(
                out=b[:], in_=staged_d[:, (K - 2 - 2 * j) * 1024 : (K - 2 * j) * 1024]
            )
            stg_f.append(f)
            stg_b.append(b)

        trans_rep = singles.tile([128, T], fp32, tag="trans_rep")
        for kk in range(4):
            nc.gpsimd.dma_start(out=trans_rep[32 * kk : 32 * (kk + 1), :], in_=trans_d)
        # rhs for the count matmuls: rhs_tc[p, c] = trans.flat[c*128 + p]
        rhs_tc = singles.tile([128, 8], fp32, tag="rhs_tc")
        tflat = trans_d.rearrange("a (b o) -> (a b) o", o=1)
        for c in range(8):
            nc.gpsimd.dma_start(out=rhs_tc[:, c : c + 1], in_=tflat[c * 128 : (c + 1) * 128, :])
        counts_sb = singles.tile([128, 8 * 128], fp32, tag="counts_sb")
        nc.gpsimd.dma_start(out=counts_sb[:], in_=counts_d)
        g_sb = singles.tile([128, L], bf16, tag="g_sb")
        nc.gpsimd.dma_start(out=g_sb[:], in_=gvals_d)

        # ---- constants -------------------------------------------------
        # X_rep[32k+i, j] = X[i, j] = exp(trans[i, j])
        x_rep = singles.tile([128, T], bf16, tag="x_rep")
        nc.scalar.activation(x_rep[:], trans_rep[:], Exp)
        # Xt_rep[32k+j, i] = X[i, j]
        xt_rep = singles.tile([128, T], bf16, tag="xt_rep")
        nc.vector.transpose(xt_rep[:], x_rep[:])

        # W_f: blockdiag lhsT for out = X @ in  (lhsT[j,i] = X[i,j])
        w_f = singles.tile([128, 128], bf16, tag="w_f")
        nc.vector.memset(w_f[:], 0.0)
        for a in range(4):
            nc.vector.tensor_copy(w_f[32 * a : 32 * a + 32, 32 * a : 32 * a + 32],
                                  xt_rep[32 * a : 32 * a + 32, :])
        # W_b: blockdiag lhsT for out = X^T @ in  (lhsT[j,i] = X[j,i])
        w_b = singles.tile([128, 128], bf16, tag="w_b")
        nc.vector.memset(w_b[:], 0.0)
        for a in range(4):
            nc.vector.tensor_copy(w_b[32 * a : 32 * a + 32, 32 * a : 32 * a + 32],
                                  x_rep[32 * a : 32 * a + 32, :])

        # ones4[32a+t, a] = 1 (partition-sum per slice)
        ones4 = singles.tile([128, 4], bf16, tag="ones4")
        nc.gpsimd.memset(ones4[:], 0.0)
        for a in range(4):
            nc.gpsimd.memset(ones4[32 * a : 32 * a + 32, a : a + 1], 1.0)
        ones41 = singles.tile([4, 1], bf16, tag="ones41")
        nc.gpsimd.memset(ones41[:], 1.0)

        identf = singles.tile([128, 128], fp32, tag="identf")
        masks.make_identity(nc, identf[:])

        bias = singles.tile([128, 1], fp32, tag="bias")
        nc.vector.memset(bias[:], -MU)

        # stopcol[32k+j] = X[STOP, j] (fp32 for tensor_scalar)
        stopcol = singles.tile([128, 1], fp32, tag="stopcol")
        nc.vector.tensor_copy(stopcol[:], xt_rep[:, STOP : STOP + 1])

        # ---- gold: count matmuls + emit reduction ----------------------
        psg = ps_pool.tile([128, 512], fp32, tag="psg")
        for c in range(8):
            nc.tensor.matmul(
                psg[0:1, 0:128],
                rhs_tc[:, c : c + 1],
                counts_sb[:, c * 128 : (c + 1) * 128],
                start=(c == 0),
                stop=(c == 7),
            )
        emit = sm_pool.tile([128, 1], fp32, tag="emit")
        nc.vector.tensor_reduce(
            emit[:], g_sb[:].rearrange("p (o l) -> p o l", o=1),
            axis=mybir.AxisListType.X, op=mybir.AluOpType.add,
        )
        # emit^T -> psg[0:1, 128:256]
        nc.tensor.matmul(psg[0:1, 128:256], emit[:], identf[:], is_transpose=True)

        # ---- exp chunks ------------------------------------------------
        fx_f = [None] * NPAIR
        fx_b = [None] * NPAIR

        def emit_exp_pair(j):
            ff = fx_pool.tile([128, DK * 1024], bf16, tag="fxf", name=f"fxf_{j}")
            nc.scalar.activation(ff[:], stg_f[j][:], Exp, bias=bias[:])
            bb = fx_pool.tile([128, DK * 1024], bf16, tag="fxb", name=f"fxb_{j}")
            nc.scalar.activation(bb[:], stg_b[j][:], Exp, bias=bias[:])
            fx_f[j] = ff
            fx_b[j] = bb

        emit_exp_pair(0)

        # ---- state init ------------------------------------------------
        st_f = st_pool.tile([128, 1024], bf16, tag="stf", name="stf_init")
        nc.vector.memset(st_f[:], 1.0)
        nc.vector.memset(st_f[0:32, 0:128], 0.0)
        # one-hot at START on the aligned [0:32] block
        nc.gpsimd.affine_select(
            out=st_f[0:32, 0:128], in_=st_f[0:32, 0:128], pattern=[[0, 128]],
            compare_op=mybir.AluOpType.not_equal, fill=1.0,
            base=-START, channel_multiplier=1,
        )

        # bwd init: w = q * fx[K-1]; q = ones except slice (7,3) = stopvec
        st_b = st_pool.tile([128, 1024], bf16, tag="stb", name="stb_init")
        nc.vector.tensor_copy(st_b[:], fx_b[0][:, 1024:2048])
        nc.vector.tensor_scalar(
            st_b[96:128, 896:1024], fx_b[0][96:128, 1024 + 896 : 1024 + 1024],
            stopcol[96:128, :], None, mult,
        )

        # ---- main loop -------------------------------------------------
        def drains(dst, src_ps, fx, off, name):
            # GPSIMD cannot touch PSUM on real hw, so the PSUM*fx multiply
            # runs on DVE only.
            nc.vector.tensor_mul(dst[:], src_ps[:], fx[:, off : off + 1024])

        mega_f = ps_pool.tile([128, 1024], fp32, tag="mega_f")
        mega_b = ps_pool.tile([128, 1024], fp32, tag="mega_b")

        for k in range(K):
            j, of = k // DK, (k % DK) * 1024
            ob = (1 - (k % DK)) * 1024
            if k % DK == 0 and j + 1 < NPAIR:
                emit_exp_pair(j + 1)
            # fwd
            for q in range(NQ):
                nc.tensor.matmul(
                    mega_f[:, q * 128 : (q + 1) * 128], w_f,
                    st_f[:, q * 128 : (q + 1) * 128], start=True, stop=True,
                )
            nst_f = st_pool.tile([128, 1024], bf16, tag="stf", name=f"stf_{k}")
            drains(nst_f, mega_f, fx_f[j], of, f"f{k}")
            st_f = nst_f
            # bwd (k=0 is the init multiply above)
            if k > 0:
                for q in range(NQ):
                    nc.tensor.matmul(
                        mega_b[:, q * 128 : (q + 1) * 128], w_b,
                        st_b[:, q * 128 : (q + 1) * 128], start=True, stop=True,
                    )
                nst_b = st_pool.tile([128, 1024], bf16, tag="stb", name=f"stb_{k}")
                drains(nst_b, mega_b, fx_b[j], ob, f"b{k}")
                st_b = nst_b

        # ---- join ------------------------------------------------------
        # y = X a_{s-1} into the slice of segment s (shifted one slice up).
        # matmul outputs must stay within one PSUM bank: <=512 fp32 cols.
        for c0 in (0, 512):
            for a in range(1, 4):
                nc.tensor.matmul(
                    mega_b[32 * a : 32 * a + 32, c0 : c0 + 512],
                    xt_rep[32 * (a - 1) : 32 * a, :],
                    st_f[32 * (a - 1) : 32 * a, c0 : c0 + 512],
                    start=True, stop=True,
                    tile_position=(32 * (a - 1), 32 * a),
                )
        # slice 0 of quad q <- slice 3 of quad q-1 (q=0 wraps; excluded later)
        for c0 in (128, 512 + 128):
            nc.tensor.matmul(
                mega_b[0:32, c0 : c0 + 384], xt_rep[96:128, :],
                st_f[96:128, c0 - 128 : c0 + 256],
                start=True, stop=True, tile_position=(96, 0),
            )
        nc.tensor.matmul(
            mega_b[0:32, 512 : 512 + 128], xt_rep[96:128, :], st_f[96:128, 384:512],
            start=True, stop=True, tile_position=(96, 0),
        )
        nc.tensor.matmul(
            mega_b[0:32, 0:128], xt_rep[96:128, :], st_f[96:128, 896:1024],
            start=True, stop=True, tile_position=(96, 0),
        )
        # z = w_s * y_{s-1}
        z_sb = sm_pool.tile([128, 1024], bf16, tag="z_sb")
        nc.vector.tensor_mul(z_sb[:], mega_b[:], st_b[:])
        # exclusion s=S-1 for the normalizer: overwrite a_{S-1} (no longer
        # needed) with 1/32 so its column sum is 1 and ln is 0
        nc.vector.memset(st_f[96:128, 896:1024], 1.0 / 32.0)
        # junction and normalizer partition-sums
        for c0 in (0, 512):
            nc.tensor.matmul(
                mega_f[0:4, c0 : c0 + 512], ones4, z_sb[:, c0 : c0 + 512],
                start=True, stop=True,
            )
            nc.tensor.matmul(
                mega_f[32:36, c0 : c0 + 512], ones4, st_f[:, c0 : c0 + 512],
                start=True, stop=True, tile_position=(0, 32),
            )
        lnj = sm_pool.tile([4, 1024], fp32, tag="lnj")
        nc.scalar.activation(lnj[:], mega_f[0:4, 0:1024], Ln)
        lnc = sm_pool.tile([4, 1024], fp32, tag="lnc")
        nc.scalar.activation(lnc[:], mega_f[32:36, 0:1024], Ln)
        # exclusion s=0: neither junction nor normalizer contribute
        diff = sm_pool.tile([4, 1024], bf16, tag="diff")
        nc.vector.tensor_sub(diff[:], lnj[:], lnc[:])
        nc.vector.memset(diff[0:1, 0:128], 0.0)
        # logZ~ (per seq) = sum over (a, q): 8 accumulating matmuls
        for q in range(NQ):
            nc.tensor.matmul(
                psg[0:1, 256:384], ones41, diff[:, q * 128 : (q + 1) * 128],
                start=(q == 0), stop=(q == 7),
            )
        # nll = logZ~ + L*mu - emit - transpart  (one PSUM operand per op)
        emt = sm_pool.tile([1, 128], fp32, tag="emt")
        nc.vector.tensor_copy(emt[:], psg[0:1, 128:256])
        t1 = sm_pool.tile([1, 128], fp32, tag="t1")
        nc.vector.tensor_sub(t1[:], psg[0:1, 256:384], emt[:])
        t2 = sm_pool.tile([1, 128], fp32, tag="t2")
        nc.vector.tensor_sub(t2[:], t1[:], psg[0:1, 0:128])
        res = sm_pool.tile([1, 128], fp32, tag="res")
        nc.vector.tensor_scalar_add(res[:], t2[:], float(L) * MU)
        nc.sync.dma_start(out=out_d, in_=res[:])

    nc.compile()
    return nc


def _stage_core(feats_c, tags_c):
    """feats_c [128, 1024, 32] f32, tags_c [128, 1024] int -> dict of arrays."""
    bf16 = ml_dtypes.bfloat16
    # staged[32a+t, k, q, b] = feats_c[b, q*128 + a*32 + k, t]
    f = np.ascontiguousarray(feats_c.transpose(1, 2, 0))  # [l, t, b]
    f = f.reshape(NQ, 4, K, T, BS)                        # [q, a, k, t, b]
    staged = np.ascontiguousarray(f.transpose(1, 3, 2, 0, 4)).reshape(128, K * NQ * BS)
    # gathered emission values
    g = np.take_along_axis(feats_c, tags_c[:, :, None].astype(np.int64), axis=2)[:, :, 0]
    # transition pair counts: pair = to*32 + from over (START+tags, tags+STOP)
    pad_start = np.concatenate(
        [np.full((BS, 1), START, tags_c.dtype), tags_c], axis=1)
    pad_stop = np.concatenate(
        [tags_c, np.full((BS, 1), STOP, tags_c.dtype)], axis=1)
    pair = (pad_stop.astype(np.int64) * T + pad_start.astype(np.int64))  # [BS, L+1]
    cnt = np.zeros((BS, T * T), np.float32)
    np.add.at(cnt, (np.arange(BS)[:, None], pair), 1.0)
    counts = np.ascontiguousarray(
        cnt.T.reshape(8, 128, BS).transpose(1, 0, 2)).reshape(128, 8 * BS)
    return {
        "staged": staged.astype(bf16),
        "gvals": g.astype(bf16),
        "counts": counts,
    }


LAST_RESULTS = None


def kernel(feats, transitions, tags, _trace=False):
    global _compiled, LAST_RESULTS
    from concourse.bass_utils import run_bass_kernel_spmd

    feats = np.asarray(feats, dtype=np.float32)
    transitions = np.asarray(transitions, dtype=np.float32)
    tags = np.asarray(tags)

    if _compiled is None:
        _compiled = _build_nc()
    nc = _compiled

    in_maps = []
    for c in range(NCORES):
        sl = slice(c * BS, (c + 1) * BS)
        m = _stage_core(feats[sl], tags[sl])
        m["trans"] = transitions
        in_maps.append(m)
    res = run_bass_kernel_spmd(
        nc, in_maps, core_ids=list(range(NCORES)), trace=_trace
    )
    LAST_RESULTS = res
    out = np.concatenate([r["out"].reshape(BS) for r in res.results])
    return out.astype(np.float32)
